# revision 45
# baseline (speedup 1.0000x reference)
"""Trainium2 Bass kernel for nn_CausalWanSelfAttention (sparse_attention).

Tensor-parallel over heads across 8 NeuronCores; each core owns 2 heads.
v2 schedule (vs baseline):
  - single-pass fused QKV projection (768-col PSUM), x loaded once
  - rope + PE transposes run on raw q,k (gains folded into the rope tables,
    which commute with the per-token rms scale) -> no AllReduce dependency
  - per-token rms scales applied late: rs_q as a broadcast multiply on rqT,
    rs_k folded into the exp's per-partition scale AP
  - attention PSUM double-buffered (oA/oB slots); block finish uses
    gpsimd partition_all_reduce + ACT exp(-ln(den)) reciprocal
  - cache-part tiles of the first two blocks run before new-part tiles so
    the rs_k chain (AllReduce-gated) is off the critical path
  - output projection emission interleaved with attention blocks
Host side (free): slicing/transposition/bf16 casts, rope freq tables with
gains folded, final concat + output bias.
"""
import sys

for _p in ("/opt/trn_rl_repo", "/root/.axon_site/_ro/trn_rl_repo"):
    if _p not in sys.path:
        sys.path.append(_p)

import numpy as np
import ml_dtypes

import concourse.bass as bass
import concourse.bacc as bacc
import concourse.mybir as mybir
from concourse import bass_isa
from concourse.tile import TileContext
from concourse.bass_utils import run_bass_kernel_spmd
from concourse.masks import make_identity

BF16 = ml_dtypes.bfloat16
S, DIM, NH, D = 1760, 2048, 16, 128
TW = 3520          # attention window length
WIN0 = 2640        # cache rows [2640:4400] form the first half of the window
NCORES, HPC = 8, 2
CH = HPC * D       # 256 channels per core
EPS = 1e-6
SCALE = 1.0 / float(np.sqrt(D))
S_OUT = S // NCORES  # 220 rows of output per core

S_TILES = [(i * 128, min(128, S - i * 128)) for i in range((S + 127) // 128)]
NT = len(S_TILES)  # 14
# window t-tiles: cache part [0,1760) then new part [1760,3520)
T_TILES = ([("c", j, off, sz) for j, (off, sz) in enumerate(S_TILES)]
           + [("n", j, off + S, sz) for j, (off, sz) in enumerate(S_TILES)])
SJ = [(0, 880), (880, 880)]  # attention query chunks

_CACHE = {}


def _emit(nc):
    dt = mybir.dt
    BF, F32 = dt.bfloat16, dt.float32
    A = mybir.ActivationFunctionType
    Op = mybir.AluOpType
    core_ids = list(range(NCORES))

    xT = nc.declare_dram_parameter("xT", [NT, 128, DIM], BF, isOutput=False)
    wT = nc.declare_dram_parameter("wT", [DIM, 3 * CH], BF, isOutput=False)
    woT = nc.declare_dram_parameter("woT", [DIM, DIM], BF, isOutput=False)
    ckT = nc.declare_dram_parameter("ckT", [HPC, D, S], BF, isOutput=False)
    cv = nc.declare_dram_parameter("cv", [HPC, 128, NT * D], BF, isOutput=False)
    # rope tables with gains folded: [q/k][4 kinds] each [NT*128 tokens, 128]
    # tiled as [128, NT*128]: [p, 128j+c] = tab[128j+p, c]
    ftab = nc.declare_dram_parameter("ftab", [8, 128, NT * 128], BF,
                                     isOutput=False)
    bqd = nc.declare_dram_parameter("bq", [1, CH], F32, isOutput=False)
    bkd = nc.declare_dram_parameter("bk", [1, CH], F32, isOutput=False)
    bvd = nc.declare_dram_parameter("bv", [1, CH], F32, isOutput=False)
    y_out = nc.declare_dram_parameter("y", [S_OUT, DIM], F32, isOutput=True)

    ss_in = [nc.dram_tensor(f"ss_in{g}", [2, 128, 7], F32) for g in range(2)]
    ss_out = [nc.dram_tensor(f"ss_out{g}", [2, 128, 7], F32, addr_space="Shared")
              for g in range(2)]
    rq_rt = nc.dram_tensor("rq_rt", [2, 1, 128, 7], BF)  # rs_q roundtrip
    # o-matrix all-to-all: two waves (s 0:880 and 880:1760) x two heads
    a2a_in = [[nc.dram_tensor(f"a2a_in{w}_{h}", [NCORES, D, 110], BF)
               for h in range(2)] for w in range(2)]
    a2a_out = [[nc.dram_tensor(f"a2a_out{w}_{h}", [NCORES, D, 110], BF)
                for h in range(2)] for w in range(2)]

    from contextlib import ExitStack
    with TileContext(nc) as tc, ExitStack() as stack:
        cpool = stack.enter_context(tc.tile_pool(name="const", bufs=1))
        wpool = stack.enter_context(tc.tile_pool(name="work", bufs=3))
        ppool = tc.alloc_tile_pool(name="projp", bufs=1)

        # ---- constants / prefetch ----
        ident = cpool.tile([128, 128], BF, tag="ident")
        make_identity(nc, ident[:])

        xt0 = wpool.tile([128, DIM], BF, tag="xt0", bufs=1, name="xt0pre")
        nc.sync.dma_start(out=xt0[:], in_=xT[0])
        xt1 = wpool.tile([128, DIM], BF, tag="xt1", bufs=1, name="xt1pre")
        nc.sync.dma_start(out=xt1[:], in_=xT[1])
        wT_sb = []
        for kk in range(16):
            t = ppool.tile([128, 3 * CH], BF, tag=f"wT{kk}", name=f"wT{kk}")
            nc.scalar.dma_start(out=t[:], in_=wT[128 * kk:128 * (kk + 1), :])
            wT_sb.append(t)

        def bcast_row(name, src):
            row = cpool.tile([1, CH], F32, tag=f"{name}_row", name=f"{name}_row")
            nc.sync.dma_start(out=row[:], in_=src[:])
            full = cpool.tile([128, CH], F32, tag=f"{name}_full", name=f"{name}_full")
            nc.gpsimd.partition_broadcast(full[:], row[:])
            return full

        bqB = bcast_row("bq", bqd)
        bkB = bcast_row("bk", bkd)
        bvB = bcast_row("bv", bvd)

        # rope tables (gains folded): order [FRE,FIE,FIO,FRO] x [q,k]
        # (on the ACT hwdge queue so they stream parallel with wT/xT)
        ftab_sb = []
        for i in range(8):
            t = ppool.tile([128, NT * 128], BF, tag=f"ft{i}", name=f"ft{i}")
            nc.scalar.dma_start(out=t[:], in_=ftab[i])
            ftab_sb.append(t)

        kwT_sb = []
        for hh in range(HPC):
            t = cpool.tile([128, TW], BF, tag=f"kwT{hh}", name=f"kwT{hh}")
            kwT_sb.append(t)
        cv_sb = [[], []]
        cv_big = []
        for hh in range(HPC):
            big = cpool.tile([128, NT * D], BF, tag=f"cva{hh}", name=f"cva{hh}")
            cv_big.append(big)
            cv_sb[hh] = [big[:, j * D:(j + 1) * D] for j in range(NT)]

        q_sb, k_sb, v_sb = [], [], []
        for j in range(NT):
            q_sb.append(ppool.tile([128, CH], BF, tag=f"q{j}", name=f"q{j}"))
            k_sb.append(ppool.tile([128, CH], BF, tag=f"k{j}", name=f"k{j}"))
            v_sb.append(cpool.tile([128, CH], BF, tag=f"v{j}", name=f"v{j}"))

        # unscaled roped q^T per (head, wave); scaled copies read by attention
        rqTu = [[cpool.tile([128, 880], BF, tag=f"rqTu{hh}_{w}",
                            name=f"rqTu{hh}_{w}") for w in range(2)]
                for hh in range(HPC)]
        rqs = [[cpool.tile([128, 880], BF, tag=f"rqs{hh}_{w}",
                           name=f"rqs{hh}_{w}") for w in range(2)]
               for hh in range(HPC)]
        oT_sb = [cpool.tile([128, S], BF, tag=f"oT{hh}", name=f"oT{hh}")
                 for hh in range(HPC)]

        HALF = [(0, 7), (7, 7)]
        ssq, ssk = [], []
        for g in range(2):
            tq = cpool.tile([128, 7], F32, tag=f"ssq{g}", name=f"ssq{g}")
            tk = cpool.tile([128, 7], F32, tag=f"ssk{g}", name=f"ssk{g}")
            nc.gpsimd.memset(tq[:], 0.0)
            nc.gpsimd.memset(tk[:], 0.0)
            ssq.append(tq)
            ssk.append(tk)

        eps_ap = cpool.tile([128, 1], F32, tag="eps_ap")
        nc.gpsimd.memset(eps_ap[:], EPS)

        # rs buffers (split per stat-group so wave-0 scaling only needs AR0)
        rsqB = [cpool.tile([128, 896], BF, tag=f"rsqB{g}", name=f"rsqB{g}")
                for g in range(2)]
        rsq_fl = [cpool.tile([1, 896], BF, tag=f"rsq_fl{g}", name=f"rsq_fl{g}")
                  for g in range(2)]
        rskT = cpool.tile([128, 14], F32, tag="rskT", name="rskT")

        def finish_ar(g):
            """rs derivation for stat-group g: no PE, no PSUM.

            rqk[p, c] = (ss/DIM + eps)^-0.5 for token 128*(7g+c)+p; q cols
            0:7, k cols 7:14. rskT (exp per-key scale) IS rqk's k columns;
            rs_q needs token-major order -> tiny DRAM roundtrip gather.
            """
            ssg = ssg_bufs[g]
            tmp = wpool.tile([128, 14], F32, tag="rstmp", name=f"rstmp{g}")
            nc.scalar.activation(tmp[:], ssg[:], A.Ln, scale=1.0 / DIM,
                                 bias=eps_ap[:])
            rqk = cpool.tile([128, 14], BF, tag=f"rqk{g}", name=f"rqk{g}")
            nc.scalar.activation(rqk[:], tmp[:], A.Exp, scale=-0.5)
            nc.scalar.activation(rskT[:, 7 * g:7 * g + 7], rqk[:, 7:14],
                                 A.Copy, scale=SCALE)
            # roundtrip on the sync HWDGE queue: the strided gather pattern
            # is expanded by hardware there, not by gpsimd ucode (which
            # burns ~15us building 896 tiny descriptors)
            nc.sync.dma_start(out=rq_rt[g][0], in_=rqk[:, 0:7])
            with nc.allow_non_contiguous_dma(reason="tiny rs_q gather (1.8KB)"):
                nc.sync.dma_start(
                    out=rsq_fl[g][0:1, :].rearrange("p (a f) -> p a f", a=7),
                    in_=rq_rt[g].rearrange("o p a -> o a p"))
            nc.gpsimd.partition_broadcast(rsqB[g][:, :], rsq_fl[g][0:1, :])

        def issue_ar(g):
            # trigger only — the (gated) readback is a separate emission so
            # it can't wedge the in-order gpsimd queue ahead of later
            # triggers/broadcasts
            nc.gpsimd.dma_start(out=ss_in[g][0], in_=ssq[g][:])
            nc.gpsimd.dma_start(out=ss_in[g][1], in_=ssk[g][:])
            nc.gpsimd.collective_compute(
                "AllReduce", mybir.AluOpType.add, replica_groups=[core_ids],
                ins=[ss_in[g][:]], outs=[ss_out[g][:]])

        def readback_ar(g):
            ssg = cpool.tile([128, 14], F32, tag=f"ssg{g}", name=f"ssg{g}")
            nc.gpsimd.dma_start(out=ssg[:, 0:7], in_=ss_out[g][0])
            nc.gpsimd.dma_start(out=ssg[:, 7:14], in_=ss_out[g][1])
            return ssg

        ssg_bufs = {}

        def rope_dve(j):
            off, sz = S_TILES[j]
            cs = slice(128 * j, 128 * j + 128)
            for qi, qk in enumerate((q_sb[j], k_sb[j])):
                tb = ftab_sb[4 * qi:4 * qi + 4]  # FRE, FIE, FIO, FRO
                q3 = qk[:sz, :].rearrange("p (h c) -> p h c", h=HPC)
                qe, qo = q3[:, :, 0:64], q3[:, :, 64:128]
                rq = ppool.tile([128, CH], BF, tag=f"rq{j}_{qi}",
                                name=f"rq{j}_{qi}")
                r3 = rq[:sz, :].rearrange("p (h c) -> p h c", h=HPC)
                t1 = wpool.tile([128, 128], BF, tag="ropet1")
                t2 = wpool.tile([128, 128], BF, tag="ropet2")
                t13 = t1[:sz, :].rearrange("p (h c) -> p h c", h=HPC)
                t23 = t2[:sz, :].rearrange("p (h c) -> p h c", h=HPC)
                f_re = tb[0][:sz, cs].rearrange("p (h c) -> p h c", h=HPC)
                f_ie = tb[1][:sz, cs].rearrange("p (h c) -> p h c", h=HPC)
                f_io = tb[2][:sz, cs].rearrange("p (h c) -> p h c", h=HPC)
                f_ro = tb[3][:sz, cs].rearrange("p (h c) -> p h c", h=HPC)
                nc.vector.tensor_mul(t13, qe, f_re)
                nc.vector.tensor_mul(t23, qo, f_ie)
                nc.vector.tensor_sub(r3[:, :, 0:64], t13, t23)
                t3 = wpool.tile([128, 128], BF, tag="ropet1")
                t4 = wpool.tile([128, 128], BF, tag="ropet2")
                t33 = t3[:sz, :].rearrange("p (h c) -> p h c", h=HPC)
                t43 = t4[:sz, :].rearrange("p (h c) -> p h c", h=HPC)
                nc.vector.tensor_mul(t33, qe, f_io)
                nc.vector.tensor_mul(t43, qo, f_ro)
                nc.vector.tensor_add(r3[:, :, 64:128], t33, t43)
                rq_store[(j, qi)] = rq

        rq_store = {}
        tr_pool_ref = []

        def rope_tr(j):
            off, sz = S_TILES[j]
            for qi in range(2):
                rq = rq_store[(j, qi)]
                for hh in range(HPC):
                    tp = tr_pool_ref[0].tile([128, 128], BF, tag="tr")
                    nc.tensor.transpose(tp[:, :sz], rq[:sz, D * hh:D * (hh + 1)],
                                        ident[:sz, :sz])
                    if qi == 1:  # k goes into the window buffer, unscaled
                        nc.vector.tensor_copy(
                            kwT_sb[hh][:, S + off:S + off + sz], tp[:, :sz])
                        continue
                    # q: split across the two 880-query wave buffers
                    if off + sz <= 880:
                        nc.vector.tensor_copy(
                            rqTu[hh][0][:, off:off + sz], tp[:, :sz])
                    elif off >= 880:
                        nc.vector.tensor_copy(
                            rqTu[hh][1][:, off - 880:off - 880 + sz],
                            tp[:, :sz])
                    else:
                        a = 880 - off
                        nc.vector.tensor_copy(
                            rqTu[hh][0][:, off:880], tp[:, :a])
                        nc.vector.tensor_copy(
                            rqTu[hh][1][:, 0:off + sz - 880], tp[:, a:sz])

        # ---- phase 1: fused QKV projection + rope + transposes ----
        with tc.tile_pool(name="pj", bufs=2, space="PSUM") as pj, \
                tc.tile_pool(name="ptr", bufs=2, space="PSUM") as ptr:
            tr_pool_ref.append(ptr)
            for j, (off, sz) in enumerate(S_TILES):
                if j == 0:
                    xt = xt0
                elif j == 1:
                    xt = xt1
                else:
                    xt = wpool.tile([128, DIM], BF, tag=f"xt{j % 2}", bufs=1,
                                    name=f"xt{j}")
                    nc.sync.dma_start(out=xt[:], in_=xT[j])
                ps = pj.tile([128, 768], F32, tag="qkv")
                for kk in range(16):
                    nc.tensor.matmul(ps[:sz, 0:512],
                                     xt[:, 128 * kk:128 * kk + sz],
                                     wT_sb[kk][:, 0:512],
                                     start=(kk == 0), stop=(kk == 15))
                    nc.tensor.matmul(ps[:sz, 512:768],
                                     xt[:, 128 * kk:128 * kk + sz],
                                     wT_sb[kk][:, 512:768],
                                     start=(kk == 0), stop=(kk == 15))
                nc.vector.tensor_add(q_sb[j][:sz, :], ps[:sz, 0:CH], bqB[:sz, :])
                nc.vector.tensor_add(k_sb[j][:sz, :], ps[:sz, CH:2 * CH],
                                     bkB[:sz, :])
                nc.vector.tensor_add(v_sb[j][:sz, :], ps[:sz, 2 * CH:3 * CH],
                                     bvB[:sz, :])
                g, col = (0, j) if j < 7 else (1, j - 7)
                sq = wpool.tile([128, CH], F32, tag="sqscratch")
                nc.scalar.activation(sq[:sz, :], q_sb[j][:sz, :], A.Square,
                                     accum_out=ssq[g][:sz, col:col + 1])
                sq2 = wpool.tile([128, CH], F32, tag="sqscratch")
                nc.scalar.activation(sq2[:sz, :], k_sb[j][:sz, :], A.Square,
                                     accum_out=ssk[g][:sz, col:col + 1])
                rope_dve(j)
                if j == 6:
                    issue_ar(0)
            ssg_bufs[0] = readback_ar(0)
            # cache loads ride the sync queue BEHIND the x tiles: needed only
            # at attention start, and keeping them off the scalar queue lets
            # wT/ftab land before the first projection tiles consume them
            for hh in range(HPC):
                nc.sync.dma_start(out=kwT_sb[hh][:, 0:S], in_=ckT[hh])
                nc.sync.dma_start(out=cv_big[hh][:], in_=cv[hh])
            # transposes after the proj matmuls so a rope stall (e.g. table
            # DMA latency) can't wedge the in-order PE stream mid-projection
            for j in range(NT):
                rope_tr(j)

        # group-0 rs chain + wave-0 q scaling (group 1 is deferred into the
        # attention phase so nothing here waits on the second AllReduce)
        finish_ar(0)
        for hh in range(HPC):
            nc.vector.tensor_mul(rqs[hh][0][:, :], rqTu[hh][0][:, :],
                                 rsqB[0][:, 0:880])

        # woT load: emitted now (transfers start once ppool's last readers
        # finish, i.e. around attention start), on the gpsimd swdge queue
        ppool.release()
        tpool = tc.alloc_tile_pool(name="tailp", bufs=1)
        woT_sb = []
        for kk in range(16):
            t = tpool.tile([128, DIM], BF, tag=f"woTf{kk}", name=f"woTf{kk}")
            nc.sync.dma_start(out=t[:], in_=woT[128 * kk:128 * (kk + 1), :])
            woT_sb.append(t)

        # ---- phase 2: attention blocks + output projection ----
        with tc.tile_pool(name="pat", bufs=1, space="PSUM") as pat:
            att = {}

            def attn_tiles(hh, jc, slot, tlist):
                jof, jsz = SJ[jc]
                st = att.get((hh, jc))
                if st is None:
                    o_ps = pat.tile([128, 880], F32, tag=f"o{slot}", bufs=1,
                                    name=f"o{hh}_{jc}")
                    den = wpool.tile([128, 880], BF, tag="den", bufs=2,
                                     name=f"den{hh}_{jc}")
                    st = att[(hh, jc)] = (o_ps, den)
                o_ps, den = st
                for ti in tlist:
                    part, j2, toff, tsz = T_TILES[ti]
                    sc = pat.tile([128, 880], F32, tag="sc", bufs=2)
                    nc.tensor.matmul(
                        sc[:tsz, 0:512], kwT_sb[hh][:, toff:toff + tsz],
                        rqs[hh][jc][:, 0:512], start=True, stop=True)
                    nc.tensor.matmul(
                        sc[:tsz, 512:880], kwT_sb[hh][:, toff:toff + tsz],
                        rqs[hh][jc][:, 512:880],
                        start=True, stop=True)
                    pT = wpool.tile([128, 880], BF, tag="pT", bufs=4)
                    if part == "c":
                        nc.scalar.activation(pT[:tsz, :], sc[:tsz, :], A.Exp,
                                             scale=SCALE)
                    else:
                        nc.scalar.activation(pT[:tsz, :], sc[:tsz, :], A.Exp,
                                             scale=rskT[:tsz, j2:j2 + 1])
                    if ti == 0:
                        nc.vector.tensor_copy(den[:, :], pT[:, :])
                    else:
                        nc.vector.tensor_add(den[:tsz, :], den[:tsz, :],
                                             pT[:tsz, :])
                    vt = (cv_sb[hh][j2][:tsz, :] if part == "c"
                          else v_sb[j2][:tsz, D * hh:D * (hh + 1)])
                    last = ti == len(T_TILES) - 1
                    nc.tensor.matmul(o_ps[:, 0:512], vt, pT[:tsz, 0:512],
                                     start=(ti == 0), stop=last)
                    nc.tensor.matmul(o_ps[:, 512:880], vt, pT[:tsz, 512:880],
                                     start=(ti == 0), stop=last)

            def attn_finish(hh, jc):
                jof, jsz = SJ[jc]
                o_ps, den = att[(hh, jc)]
                denf = wpool.tile([128, 880], F32, tag="denf", bufs=2,
                                  name=f"denf{hh}_{jc}")
                nc.gpsimd.partition_all_reduce(denf[:, :], den[:, :], 128,
                                               bass_isa.ReduceOp.add)
                # reciprocal on DVE: slower than ACT exp(-ln) but off the
                # in-order ACT exp stream, and avoids the two ACT table
                # reloads per finish that were stalling it (v10 trace)
                denr = wpool.tile([128, 880], F32, tag="denr", bufs=2,
                                  name=f"denr{hh}_{jc}")
                nc.vector.reciprocal(denr[:, :jsz], denf[:, :jsz])
                nc.vector.tensor_mul(
                    oT_sb[hh][:, jof:jof + jsz], o_ps[:, :jsz], denr[:, :jsz])

            def emit_a2a(w, hh):
                nc.sync.dma_start(
                    out=a2a_in[w][hh][:].rearrange("d p s -> p d s"),
                    in_=oT_sb[hh][:, 880 * w:880 * (w + 1)]
                        .rearrange("p (d s) -> p d s", s=110))
                nc.gpsimd.collective_compute(
                    "AllToAll", mybir.AluOpType.bypass,
                    replica_groups=[core_ids],
                    ins=[a2a_in[w][hh][:]], outs=[a2a_out[w][hh][:]])

            y_state = {}

            def load_otr(w, hh):
                otr, _, _ = y_state[w]
                t = tpool.tile([128, 8 * 110], BF, tag=f"otr{w}_{hh}",
                               name=f"otr{w}_{hh}")
                nc.sync.dma_start(
                    out=t[:].rearrange("p (d s) -> p d s", s=110),
                    in_=a2a_out[w][hh][:].rearrange("d p s -> p d s"))
                otr[hh] = t

            def wave_y_pre(w, slot, heads=(0, 1)):
                yf = wpool.tile([128, DIM], F32, tag="yf", bufs=1, name=f"yf{w}")
                y_state[w] = ([None, None], yf, slot)
                for hh in heads:
                    load_otr(w, hh)

            def wave_y_chunk(w, n):
                otr, yf, slot = y_state[w]
                yp = pat.tile([128, 880], F32, tag=f"o{slot}", bufs=1,
                              name=f"yp{w}_{n}")
                for kk in range(16):
                    src_c, hh = kk // 2, kk % 2
                    nc.tensor.matmul(
                        yp[:110, 0:512],
                        otr[hh][:, 110 * src_c:110 * (src_c + 1)],
                        woT_sb[kk][:, 512 * n:512 * (n + 1)],
                        start=(kk == 0), stop=(kk == 15))
                nc.vector.tensor_copy(yf[:110, 512 * n:512 * (n + 1)],
                                      yp[:110, 0:512])

            def wave_y_out(w):
                _, yf, _ = y_state[w]
                nc.sync.dma_start(out=y_out[110 * w:110 * (w + 1), :],
                                  in_=yf[:110, :])

            # block order: cache parts of (0,0),(1,0) first (rs_k-independent)
            attn_tiles(0, 0, "A", range(0, 14))
            attn_tiles(1, 0, "B", range(0, 14))
            # group-1 AllReduce issued here: its blocking gpsimd trigger
            # then can't stall the group-0 chain, and the 28 cache-part
            # tiles above cover its latency
            issue_ar(1)
            ssg_bufs[1] = readback_ar(1)
            finish_ar(1)
            # wave 1 q scale: tokens 880:896 from group 0, rest from group 1
            for hh in range(HPC):
                nc.vector.tensor_mul(rqs[hh][1][:, 0:16],
                                     rqTu[hh][1][:, 0:16],
                                     rsqB[0][:, 880:896])
                nc.vector.tensor_mul(rqs[hh][1][:, 16:880],
                                     rqTu[hh][1][:, 16:880],
                                     rsqB[1][:, 0:864])
            attn_tiles(0, 0, "A", range(14, 28))
            attn_finish(0, 0)
            emit_a2a(0, 0)
            attn_tiles(1, 0, "B", range(14, 28))
            attn_finish(1, 0)
            emit_a2a(0, 1)
            # wave 1 blocks, with wave-0 y-proj interleaved
            wave_y_pre(0, "B")
            attn_tiles(0, 1, "A", range(0, 7))
            wave_y_chunk(0, 0)
            attn_tiles(0, 1, "A", range(7, 14))
            wave_y_chunk(0, 1)
            attn_tiles(0, 1, "A", range(14, 21))
            wave_y_chunk(0, 2)
            attn_tiles(0, 1, "A", range(21, 28))
            wave_y_chunk(0, 3)
            attn_finish(0, 1)
            emit_a2a(1, 0)
            wave_y_out(0)
            attn_tiles(1, 1, "B", range(28))
            # y1: head-0 half pre-accumulated (closed matmul groups) while
            # b4's finish chain and the final all-to-all drain; after the
            # a2a only the head-1 half + a DVE merge remain
            wave_y_pre(1, "A", heads=(0,))
            otr1, yf1, _ = y_state[1]
            yh0 = wpool.tile([128, DIM], F32, tag="yh0", bufs=1, name="yh0")
            for n in range(4):
                yp = pat.tile([128, 880], F32, tag="oA", bufs=1,
                              name=f"y1a{n}")
                for i, kk in enumerate(range(0, 16, 2)):
                    nc.tensor.matmul(
                        yp[:110, 0:512],
                        otr1[0][:, 110 * (kk // 2):110 * (kk // 2 + 1)],
                        woT_sb[kk][:, 512 * n:512 * (n + 1)],
                        start=(i == 0), stop=(i == 7))
                nc.vector.tensor_copy(yh0[:110, 512 * n:512 * (n + 1)],
                                      yp[:110, 0:512])
            attn_finish(1, 1)
            emit_a2a(1, 1)
            load_otr(1, 1)
            for n in range(4):
                yp = pat.tile([128, 880], F32, tag="oA", bufs=1,
                              name=f"y1b{n}")
                for i, kk in enumerate(range(1, 16, 2)):
                    nc.tensor.matmul(
                        yp[:110, 0:512],
                        otr1[1][:, 110 * (kk // 2):110 * (kk // 2 + 1)],
                        woT_sb[kk][:, 512 * n:512 * (n + 1)],
                        start=(i == 0), stop=(i == 7))
                nc.vector.tensor_add(yf1[:110, 512 * n:512 * (n + 1)],
                                     yp[:110, 0:512],
                                     yh0[:110, 512 * n:512 * (n + 1)])
            wave_y_out(1)
        tpool.release()


def _build():
    if "nc" not in _CACHE:
        nc = bacc.Bacc("TRN2", target_bir_lowering=False, debug=False,
                       num_devices=NCORES)
        _emit(nc)
        nc.compile()
        _CACHE["nc"] = nc
    return _CACHE["nc"]


def _make_fcomb(freqs):
    F, H, W = 2, 20, 44
    fr = np.asarray(freqs, np.float32)  # [1024, 64, 2]
    fpart = np.broadcast_to(fr[5:7, None, None, 0:22], (F, H, W, 22, 2))
    hpart = np.broadcast_to(fr[None, 0:H, None, 22:43], (F, H, W, 21, 2))
    wpart = np.broadcast_to(fr[None, None, 0:W, 43:64], (F, H, W, 21, 2))
    return np.concatenate([fpart, hpart, wpart], axis=3).reshape(S, 64, 2)


def kernel(x, wq, bq, wk, bk, wv, bv, wo, bo, gq, gk, freqs, cache_k, cache_v):
    x = np.asarray(x, np.float32)
    wq, wk, wv, wo = (np.asarray(a, np.float32) for a in (wq, wk, wv, wo))
    bq, bk, bv, bo = (np.asarray(a, np.float32) for a in (bq, bk, bv, bo))
    gq, gk = np.asarray(gq, np.float32), np.asarray(gk, np.float32)
    cache_k = np.asarray(cache_k, np.float32)
    cache_v = np.asarray(cache_v, np.float32)

    fcomb = _make_fcomb(freqs)  # [S, 64, 2]
    fr_t, fi_t = fcomb[..., 0], fcomb[..., 1]  # [S, 64]
    # pre-tiled x^T: xT[j, p, kk*128+c] = x[128j+c, 128kk+p]
    xp = np.zeros((NT * 128, DIM), np.float32)
    xp[:S] = x[0]
    xT = np.ascontiguousarray(
        xp.reshape(NT, 128, 16, 128).transpose(0, 3, 2, 1).reshape(NT, 128, DIM)
    ).astype(BF16)

    # de-interleave rope channel pairs within each head: [2c] then [2c+1]
    perm = np.concatenate([np.arange(0, D, 2), np.arange(1, D, 2)])
    qk_perm = np.concatenate([h * D + perm for h in range(NH)])
    wqp, wkp = wq[qk_perm], wk[qk_perm]
    bqp, bkp = bq[qk_perm], bk[qk_perm]
    gqp, gkp = gq[qk_perm], gk[qk_perm]
    ck_perm = cache_k[0, WIN0:WIN0 + S][:, :, perm]  # [S, NH, D] permuted

    woT_full = np.ascontiguousarray(wo.T).astype(BF16)  # [DIM, DIM]

    def tile_tab(tab):
        # [S,128] -> [128, NT*128]: [p, 128j+c] = tab[128j+p, c]
        tp = np.zeros((NT * 128, 128), np.float32)
        tp[:S] = tab
        return tp.reshape(NT, 128, 128).transpose(1, 0, 2).reshape(128, NT * 128)

    in_maps = []
    for c in range(NCORES):
        hs = slice(CH * c, CH * (c + 1))
        h0 = HPC * c
        wTc = np.concatenate([wqp[hs].T, wkp[hs].T, wv[hs].T], axis=1).astype(BF16)
        ckTc = np.ascontiguousarray(
            ck_perm[:, h0:h0 + HPC, :].transpose(1, 2, 0)
        ).astype(BF16)  # [HPC, D, S]
        cw = np.zeros((NT * 128, HPC, D), np.float32)
        cw[:S] = cache_v[0, WIN0:WIN0 + S, h0:h0 + HPC, :]
        cvc = np.ascontiguousarray(
            cw.reshape(NT, 128, HPC, D).transpose(2, 1, 0, 3).reshape(HPC, 128, NT * D)
        ).astype(BF16)
        # rope tables with per-head gains folded: col layout h*64+cc
        tabs = []
        for gv in (gqp, gkp):
            g2 = gv[hs].reshape(HPC, 128)  # per head: [e(64) | o(64)]
            ge = np.concatenate([g2[h, 0:64] for h in range(HPC)])  # [128]
            go = np.concatenate([g2[h, 64:128] for h in range(HPC)])
            f_r = np.tile(fr_t, (1, HPC))  # [S, 128]
            f_i = np.tile(fi_t, (1, HPC))
            tabs.append(tile_tab(f_r * ge[None, :]))   # FRE
            tabs.append(tile_tab(f_i * go[None, :]))   # FIE
            tabs.append(tile_tab(f_i * ge[None, :]))   # FIO
            tabs.append(tile_tab(f_r * go[None, :]))   # FRO
        ftabc = np.ascontiguousarray(np.stack(tabs)).astype(BF16)
        in_maps.append({
            "xT": xT, "wT": np.ascontiguousarray(wTc), "woT": woT_full,
            "ckT": ckTc, "cv": cvc, "ftab": ftabc,
            "bq": np.ascontiguousarray(bqp[hs])[None, :],
            "bk": np.ascontiguousarray(bkp[hs])[None, :],
            "bv": np.ascontiguousarray(bv[hs])[None, :],
        })

    nc = _build()
    res = run_bass_kernel_spmd(nc, in_maps, list(range(NCORES)))
    _CACHE["last_result"] = res
    # all-to-all layout: core c returns rows [110c:110c+110] and
    # [880+110c:880+110c+110]
    y = np.empty((S, DIM), np.float32)
    for c in range(NCORES):
        yc = res.results[c]["y"]
        y[110 * c:110 * (c + 1)] = yc[:110]
        y[880 + 110 * c:880 + 110 * (c + 1)] = yc[110:]
    return (y + bo[None, :]).reshape(1, S, DIM).astype(np.float32)
